# revision 4
# baseline (speedup 1.0000x reference)
"""DeltaNet-style gated linear attention forward on 8 Trainium2 NeuronCores.

Sharding: core c handles batch b = c // 2 and heads h0..h0+3 with
h0 = 4 * (c % 2) (data-parallel over batch, head-parallel over heads; the
recurrent DxD state is independent per (b, h)).  Each core computes a partial
output (its 4 heads' contribution through the output projection); the host
sums the two partials per batch and adds bout.

Per-core device pipeline (token tiles of 512 = 4 chunks of 128):
  RMSNorm (norm_w folded into weights on host) -> transpose h to feature-major
  -> q/k projections (feature-major) + RoPE -> v projection (token-major)
  -> gate projections + sigmoid (via exp, staying in one ACT table set)
  -> per-chunk decay cumsum via a triangular matmul
  -> chunked delta-rule scan entirely via PE matmuls
  -> output projection (partial over this core's 512 head-dims).
"""

import sys
import types
import math

import numpy as np

sys.path.insert(0, "/opt/trn_rl_repo")

B, L, HID, NH, DH = 4, 4096, 1024, 8, 128
NHPC = 4              # heads per core
DPC = NHPC * DH       # 512 head dims per core
TT = 512              # tokens per tile
NTT = L // TT         # 8 token tiles
C = 128               # chunk length (= one SBUF partition block)
SC = TT // C          # 4 chunks per token tile
THETA = 10000.0
SCALE = 1.0 / math.sqrt(DH)
EPS = 1e-6

_CACHE = {}


def _install_ntff_shim():
    """Register the axon NTFF profile hook (image antenv lacks axon_hooks)."""
    import concourse.bass_utils as bu

    bu.upload_artifacts = lambda tmpdir: "local://" + tmpdir
    if "antenv.axon_hooks" in sys.modules:
        return
    try:
        from trn_agent_boot.trn_boot import _ntff_profile_via_ctypes

        hook = _ntff_profile_via_ctypes("/opt/axon/libaxon_pjrt.so")
    except Exception:
        hook = None
    import antenv

    mod = types.ModuleType("antenv.axon_hooks")
    mod.get_axon_ntff_profile_hook = lambda: hook
    mod.set_axon_ntff_profile_hook = lambda h: None
    sys.modules["antenv.axon_hooks"] = mod
    antenv.axon_hooks = mod


def _build_nc():
    import concourse.bass as bass
    import concourse.tile as tile
    from concourse import mybir, bacc

    F32 = mybir.dt.float32
    AF = mybir.ActivationFunctionType
    OP = mybir.AluOpType

    nc = bacc.Bacc("TRN2", target_bir_lowering=False, debug=False, num_devices=8)

    xb = nc.dram_tensor("xb", [L, HID], F32, kind="ExternalInput")
    wqt = nc.dram_tensor("wqt", [HID, DPC], F32, kind="ExternalInput")
    wkt = nc.dram_tensor("wkt", [HID, DPC], F32, kind="ExternalInput")
    wvt = nc.dram_tensor("wvt", [HID, DPC], F32, kind="ExternalInput")
    wgt = nc.dram_tensor("wgt", [HID, 12], F32, kind="ExternalInput")
    bg = nc.dram_tensor("bg", [1, 12], F32, kind="ExternalInput")
    wot = nc.dram_tensor("wot", [DPC, HID], F32, kind="ExternalInput")
    cst = nc.dram_tensor("cst", [128, L], F32, kind="ExternalInput")  # [cos;sin]
    sct = nc.dram_tensor("sct", [128, L], F32, kind="ExternalInput")  # [sin;cos]
    triu_d = nc.dram_tensor("triu", [128, 128], F32, kind="ExternalInput")
    ones_d = nc.dram_tensor("ones", [128, 128], F32, kind="ExternalInput")
    iden_d = nc.dram_tensor("iden", [128, 128], F32, kind="ExternalInput")
    out = nc.dram_tensor("out", [L, HID], F32, kind="ExternalOutput")

    with tile.TileContext(nc) as tc:
        with (
            tc.tile_pool(name="consts", bufs=1) as consts,
            tc.tile_pool(name="xin", bufs=2) as xin,
            tc.tile_pool(name="ht", bufs=2) as htp,
            tc.tile_pool(name="qkv", bufs=2) as qkvp,
            tc.tile_pool(name="rope", bufs=2) as ropep,
            tc.tile_pool(name="gates", bufs=3) as gatep,
            tc.tile_pool(name="scan", bufs=4) as scanp,
            tc.tile_pool(name="osb", bufs=2) as osbp,
            tc.tile_pool(name="psum", bufs=1, space="PSUM") as psump,
        ):
            # ---- constants / weights resident in SBUF ----
            wq_sb = consts.tile([128, 8, DPC], F32, tag="wq")
            nc.sync.dma_start(out=wq_sb[:], in_=wqt[:].rearrange("(n p) e -> p n e", p=128))
            wk_sb = consts.tile([128, 8, DPC], F32, tag="wk")
            nc.sync.dma_start(out=wk_sb[:], in_=wkt[:].rearrange("(n p) e -> p n e", p=128))
            wv_sb = consts.tile([128, 8, DPC], F32, tag="wv")
            nc.sync.dma_start(out=wv_sb[:], in_=wvt[:].rearrange("(n p) e -> p n e", p=128))
            wg_sb = consts.tile([128, 8, 12], F32, tag="wg")
            nc.sync.dma_start(out=wg_sb[:], in_=wgt[:].rearrange("(n p) e -> p n e", p=128))
            wo_sb = consts.tile([128, NHPC, HID], F32, tag="wo")
            nc.sync.dma_start(out=wo_sb[:], in_=wot[:].rearrange("(h p) e -> p h e", p=128))
            triu_sb = consts.tile([128, 128], F32, tag="triu")
            nc.sync.dma_start(out=triu_sb[:], in_=triu_d[:])
            ones_sb = consts.tile([128, 128], F32, tag="ones")
            nc.sync.dma_start(out=ones_sb[:], in_=ones_d[:])
            iden_sb = consts.tile([128, 128], F32, tag="iden")
            nc.sync.dma_start(out=iden_sb[:], in_=iden_d[:])
            bg_sb = consts.tile([128, 12], F32, tag="bg")
            bg_ap = bg[:]
            bg_bcast = bass.AP(tensor=bg_ap.tensor, offset=bg_ap.offset,
                               ap=[[0, 128]] + list(bg_ap.ap[1:]))
            nc.gpsimd.dma_start(out=bg_sb[:], in_=bg_bcast)

            eps_sb = consts.tile([128, 1], F32, tag="eps")
            nc.vector.memset(eps_sb[:], EPS)

            # recurrent state, one [DH, DH] block per head
            m_sb = consts.tile([128, NHPC, DH], F32, tag="mstate")
            nc.vector.memset(m_sb[:], 0.0)

            for tt in range(NTT):
                t0 = tt * TT
                # RoPE table slices for this token tile
                cs_sb = ropep.tile([128, TT], F32, tag="cs")
                nc.sync.dma_start(out=cs_sb[:], in_=cst[:, t0 : t0 + TT])
                sc_sb = ropep.tile([128, TT], F32, tag="sc")
                nc.sync.dma_start(out=sc_sb[:], in_=sct[:, t0 : t0 + TT])

                # hT: feature-major normalized input, [d_part, chunk, d_tile, tok]
                ht_sb = htp.tile([128, SC, 8, C], F32, tag="ht")
                # per-chunk gate vectors for the scan (stacked over chunks)
                oecl_t = gatep.tile([128, SC, NHPC], F32, tag="oecl", bufs=2)
                vscale_t = gatep.tile([128, SC, NHPC], F32, tag="vscale", bufs=2)
                kscale_t = gatep.tile([128, SC, NHPC], F32, tag="kscale", bufs=2)
                eclc_t = gatep.tile([128, SC, NHPC], F32, tag="eclc", bufs=2)

                # ---- RMSNorm + transpose + gates, one 128-token chunk at a time ----
                for s in range(SC):
                    x_t = xin.tile([128, HID], F32, tag="x")
                    nc.sync.dma_start(out=x_t[:], in_=xb[t0 + s * C : t0 + (s + 1) * C, :])
                    sq = xin.tile([128, HID], F32, tag="sq", bufs=1)
                    ss = gatep.tile([128, 1], F32, tag="ss")
                    nc.scalar.activation(out=sq[:], in_=x_t[:], func=AF.Square,
                                         accum_out=ss[:])
                    lnv = gatep.tile([128, 1], F32, tag="lnv")
                    nc.scalar.activation(out=lnv[:], in_=ss[:], func=AF.Ln,
                                         scale=1.0 / HID, bias=eps_sb[:])
                    invr = gatep.tile([128, 1], F32, tag="invr")
                    nc.scalar.activation(out=invr[:], in_=lnv[:], func=AF.Exp, scale=-0.5)
                    nc.vector.tensor_scalar_mul(out=x_t[:], in0=x_t[:], scalar1=invr[:])

                    # transpose h chunk into feature-major via PE
                    pt_a = psump.tile([128, 512], F32, tag="ptrans", bufs=2)
                    pt_b = psump.tile([128, 512], F32, tag="ptrans", bufs=2)
                    for dt in range(8):
                        dst = pt_a if dt < 4 else pt_b
                        col = (dt % 4) * 128
                        nc.tensor.transpose(dst[:, col : col + 128],
                                            x_t[:, dt * 128 : (dt + 1) * 128],
                                            iden_sb[:])
                    nc.scalar.copy(out=ht_sb[:, s, 0:4, :],
                                   in_=pt_a[:].rearrange("p (n t) -> p n t", n=4))
                    nc.scalar.copy(out=ht_sb[:, s, 4:8, :],
                                   in_=pt_b[:].rearrange("p (n t) -> p n t", n=4))

                    # ---- gates for this chunk ----
                    gl_ps = psump.tile([128, 12], F32, tag="psc", bufs=3)
                    for dt in range(8):
                        nc.tensor.matmul(gl_ps[:], ht_sb[:, s, dt, :], wg_sb[:, dt, :],
                                         start=(dt == 0), stop=(dt == 7))
                    zb = gatep.tile([128, 12], F32, tag="zb")
                    nc.vector.tensor_add(zb[:], gl_ps[:], bg_sb[:])
                    en = gatep.tile([128, 12], F32, tag="en")
                    nc.scalar.activation(out=en[:], in_=zb[:], func=AF.Exp, scale=-1.0)
                    den = gatep.tile([128, 12], F32, tag="den")
                    nc.vector.tensor_scalar_add(out=den[:], in0=en[:], scalar1=1.0)
                    gts = gatep.tile([128, 12], F32, tag="gts")
                    nc.vector.reciprocal(out=gts[:], in_=den[:])

                    logf = gatep.tile([128, 4], F32, tag="logf")
                    nc.scalar.activation(out=logf[:], in_=gts[:, 0:4], func=AF.Ln)
                    cl_ps = psump.tile([128, 4], F32, tag="psc", bufs=3)
                    nc.tensor.matmul(cl_ps[:], triu_sb[:], logf[:], start=True, stop=True)
                    ecl = gatep.tile([128, 4], F32, tag="ecl")
                    nc.scalar.activation(out=ecl[:], in_=cl_ps[:], func=AF.Exp)
                    emcl = gatep.tile([128, 4], F32, tag="emcl")
                    nc.scalar.activation(out=emcl[:], in_=cl_ps[:], func=AF.Exp, scale=-1.0)
                    nc.vector.tensor_mul(oecl_t[:, s, :], ecl[:], gts[:, 8:12])
                    nc.vector.scalar_tensor_tensor(out=vscale_t[:, s, :], in0=emcl[:],
                                                   scalar=SCALE, in1=gts[:, 4:8],
                                                   op0=OP.mult, op1=OP.mult)
                    sf_ps = psump.tile([128, 4], F32, tag="psc", bufs=3)
                    nc.tensor.matmul(sf_ps[:], ones_sb[:], logf[:], start=True, stop=True)
                    nc.scalar.activation(out=eclc_t[:, s, :], in_=sf_ps[:], func=AF.Exp)
                    nc.vector.tensor_mul(kscale_t[:, s, :], vscale_t[:, s, :],
                                         eclc_t[:, s, :])

                # ---- q/k projections + RoPE (feature-major) ----
                qf_sb = qkvp.tile([128, NHPC, TT], F32, tag="qf")
                kf_sb = qkvp.tile([128, NHPC, TT], F32, tag="kf")
                for (wsb, dst) in ((wq_sb, qf_sb), (wk_sb, kf_sb)):
                    for hh in range(NHPC):
                        pp = psump.tile([128, 512], F32, tag="pbig", bufs=3)
                        for dt in range(8):
                            nc.tensor.matmul(pp[:], wsb[:, dt, hh * 128 : (hh + 1) * 128],
                                             ht_sb[:, :, dt, :],
                                             start=(dt == 0), stop=(dt == 7))
                        qs = ropep.tile([128, TT], F32, tag="ropein")
                        nc.scalar.copy(out=qs[:], in_=pp[:])
                        a1 = ropep.tile([64, TT], F32, tag="a1")
                        nc.vector.tensor_mul(a1[:], qs[0:64, :], cs_sb[0:64, :])
                        a2 = ropep.tile([64, TT], F32, tag="a2")
                        nc.vector.tensor_mul(a2[:], qs[64:128, :], cs_sb[64:128, :])
                        nc.vector.tensor_sub(dst[0:64, hh, :], a1[:], a2[:])
                        a3 = ropep.tile([64, TT], F32, tag="a3")
                        nc.vector.tensor_mul(a3[:], qs[0:64, :], sc_sb[0:64, :])
                        a4 = ropep.tile([64, TT], F32, tag="a4")
                        nc.vector.tensor_mul(a4[:], qs[64:128, :], sc_sb[64:128, :])
                        nc.vector.tensor_add(dst[64:128, hh, :], a3[:], a4[:])

                # ---- v projection (token-major) ----
                v_sb = qkvp.tile([128, SC, DPC], F32, tag="v")
                for s in range(SC):
                    pv = psump.tile([128, 512], F32, tag="pbig", bufs=3)
                    for dt in range(8):
                        nc.tensor.matmul(pv[:], ht_sb[:, s, dt, :], wv_sb[:, dt, :],
                                         start=(dt == 0), stop=(dt == 7))
                    nc.scalar.copy(out=v_sb[:, s, :], in_=pv[:])

                # ---- chunked scan + output projection ----
                for s in range(SC):
                    yf_tiles = []
                    for hh in range(NHPC):
                        qfc = qf_sb[:, hh, s * C : (s + 1) * C]
                        kfc = kf_sb[:, hh, s * C : (s + 1) * C]
                        vc = v_sb[:, s, hh * 128 : (hh + 1) * 128]

                        s_ps = psump.tile([128, 128], F32, tag="psc", bufs=3)
                        nc.tensor.matmul(s_ps[:], kfc, qfc, start=True, stop=True)
                        sm = scanp.tile([128, 128], F32, tag="sm")
                        nc.vector.scalar_tensor_tensor(out=sm[:], in0=s_ps[:],
                                                       scalar=vscale_t[:, s, hh : hh + 1],
                                                       in1=triu_sb[:],
                                                       op0=OP.mult, op1=OP.mult)

                        kt_ps = psump.tile([128, 128], F32, tag="psc", bufs=3)
                        nc.tensor.transpose(kt_ps[:], kfc, iden_sb[:])
                        kts = scanp.tile([128, 128], F32, tag="kts")
                        nc.vector.tensor_scalar_mul(out=kts[:], in0=kt_ps[:],
                                                    scalar1=kscale_t[:, s, hh : hh + 1])

                        y_ps = psump.tile([128, 128], F32, tag="psc", bufs=3)
                        nc.tensor.matmul(y_ps[:], sm[:], vc, start=True, stop=False)
                        nc.tensor.matmul(y_ps[:], qfc, m_sb[:, hh, :], start=False, stop=True)
                        y_tm = scanp.tile([128, 128], F32, tag="ytm")
                        nc.vector.tensor_scalar_mul(out=y_tm[:], in0=y_ps[:],
                                                    scalar1=oecl_t[:, s, hh : hh + 1])

                        mu_ps = psump.tile([128, 128], F32, tag="psc", bufs=3)
                        nc.tensor.matmul(mu_ps[:], kts[:], vc, start=True, stop=True)
                        nc.vector.scalar_tensor_tensor(out=m_sb[:, hh, :], in0=m_sb[:, hh, :],
                                                       scalar=eclc_t[:, s, hh : hh + 1],
                                                       in1=mu_ps[:],
                                                       op0=OP.mult, op1=OP.add)

                        yt_ps = psump.tile([128, 128], F32, tag="psc", bufs=3)
                        nc.tensor.transpose(yt_ps[:], y_tm[:], iden_sb[:])
                        yf = scanp.tile([128, 128], F32, tag="yf")
                        nc.scalar.copy(out=yf[:], in_=yt_ps[:])
                        yf_tiles.append(yf)

                    # output projection for this chunk (partial over 4 heads)
                    o_sb = osbp.tile([128, HID], F32, tag="osb")
                    for half in range(2):
                        po = psump.tile([128, 512], F32, tag="pbig", bufs=3)
                        for hh in range(NHPC):
                            nc.tensor.matmul(po[:], yf_tiles[hh][:],
                                             wo_sb[:, hh, half * 512 : (half + 1) * 512],
                                             start=(hh == 0), stop=(hh == 3))
                        nc.vector.tensor_copy(out=o_sb[:, half * 512 : (half + 1) * 512],
                                              in_=po[:])
                    nc.sync.dma_start(out=out[t0 + s * C : t0 + (s + 1) * C, :], in_=o_sb[:])

    nc.finalize()
    return nc


def _host_prep(inputs):
    """Build per-core input maps from the full problem inputs."""
    x = np.asarray(inputs["x"], np.float32)
    norm_w = np.asarray(inputs["norm_w"], np.float32)
    Wq = np.asarray(inputs["Wq"], np.float32)
    Wk = np.asarray(inputs["Wk"], np.float32)
    Wv = np.asarray(inputs["Wv"], np.float32)
    Wbeta = np.asarray(inputs["Wbeta"], np.float32)
    bbeta = np.asarray(inputs["bbeta"], np.float32)
    Wig = np.asarray(inputs["Wig"], np.float32)
    big = np.asarray(inputs["big"], np.float32)
    Wog = np.asarray(inputs["Wog"], np.float32)
    bog = np.asarray(inputs["bog"], np.float32)
    Wout = np.asarray(inputs["Wout"], np.float32)

    half = DH // 2
    inv_freq = 1.0 / (THETA ** (np.arange(half, dtype=np.float32) / half))
    tpos = np.arange(L, dtype=np.float32)
    ang = tpos[None, :] * inv_freq[:, None]          # [64, L] feature-major
    cosf = np.cos(ang).astype(np.float32)
    sinf = np.sin(ang).astype(np.float32)
    cst = np.concatenate([cosf, sinf], 0)            # [128, L]
    sct = np.concatenate([sinf, cosf], 0)
    triu = np.triu(np.ones((128, 128), np.float32))
    iden = np.eye(128, dtype=np.float32)

    in_maps = []
    for c in range(8):
        b = c // 2
        h0 = 4 * (c % 2)
        r0, r1 = h0 * DH, (h0 + NHPC) * DH
        wgrows = np.concatenate([Wbeta[h0 : h0 + 4], Wig[h0 : h0 + 4],
                                 Wog[h0 : h0 + 4]], 0)          # [12, HID]
        bgv = np.concatenate([bbeta[h0 : h0 + 4], big[h0 : h0 + 4],
                              bog[h0 : h0 + 4]], 0)             # [12]
        in_maps.append({
            "xb": np.ascontiguousarray(x[b]),
            "wqt": np.ascontiguousarray((Wq[r0:r1] * norm_w[None, :]).T),
            "wkt": np.ascontiguousarray((Wk[r0:r1] * norm_w[None, :]).T),
            "wvt": np.ascontiguousarray((Wv[r0:r1] * norm_w[None, :]).T),
            "wgt": np.ascontiguousarray((wgrows * norm_w[None, :]).T),
            "bg": np.ascontiguousarray(bgv[None, :]),
            "wot": np.ascontiguousarray(Wout[:, r0:r1].T),
            "cst": cst,
            "sct": sct,
            "triu": triu,
            "ones": np.ones((128, 128), np.float32),
            "iden": iden,
        })
    return in_maps


def kernel(**inputs):
    _install_ntff_shim()
    from concourse.bass_utils import run_bass_kernel_spmd

    if "nc" not in _CACHE:
        _CACHE["nc"] = _build_nc()
    nc = _CACHE["nc"]

    in_maps = _host_prep(inputs)
    trace = bool(_CACHE.get("trace", False))
    res = run_bass_kernel_spmd(nc, in_maps, core_ids=list(range(8)), trace=trace)
    _CACHE["last_result"] = res

    bout = np.asarray(inputs["bout"], np.float32)
    out = np.empty((B, L, HID), np.float32)
    for b in range(B):
        out[b] = res.results[2 * b]["out"] + res.results[2 * b + 1]["out"] + bout[None, :]
    return out
